# revision 22
# baseline (speedup 1.0000x reference)
"""Multi-head self-attention (RoPE, causal) on 8 Trainium2 NeuronCores.

Sharding: core c -> (batch = c//2, head-group = c%2 of 8 heads).
Column-parallel wq/wk/wv, row-parallel wo. Each core emits a partial
out^T [f, s]; the host sums the two partials per batch and transposes.

Layouts (all chosen so no on-device transposes are needed):
  XT  [d, s]   (x transposed on host, bf16)
  Q^T/K^T [e, s] per head from matmul(lhsT=wT[d,e], rhs=XT[d,s])
  V   [s, e]   from matmul(lhsT=XT[d,s], rhs=wvT[d,e])
  S^T [j, i] = matmul(lhsT=K^T[e,j], rhs=Q^T[e,i])
  ctx^T [e, i] = matmul(lhsT=V[j,e], rhs=expS^T[j,i])
  out^T [f, s] = matmul(lhsT=woT[d,f], rhs=ctx^T[d,s])

All DRAM inputs are pre-tiled on the host so every DMA moves dense,
multi-KB contiguous per-partition lines (<=2KB lines run at ~1/2 DMA
rate). All matmul operands are bf16 (PSUM accumulation stays fp32);
softmax statistics and RoPE arithmetic stay fp32.

RoPE: head dims de-interleaved on host (even dims -> partitions 0..63,
odd -> 64..127 of each head's Q^T/K^T) by permuting wq/wk rows. Then
rot(x) = x*cc + swap(x)*ss_signed where swap is a partition-half
rotation done by two SBUF->SBUF DMAs (free on the tensor engine) and
the pair-rotation sign lives in the host-built ss table. The
1/sqrt(dk) scale is applied via the Exp activation's scale field.

Softmax: no max-subtraction (scores are O(1)-scaled; fp32 exp is safe).
Causal masking by block-skipping + one 128x128 triangular mask on
diagonal blocks. Row sums via an all-ones [128,128] matmul on a
DVE-accumulated sum of the i-block's exp tiles (output rows all equal
the row sum, giving the partition broadcast for free); normalization
multiplies ctx^T by a fast DVE reciprocal of that tile.

The tensor engine is the bottleneck (~94% busy): ~2700 matmul
instructions at the 512-column bf16 streaming rate. This version
removes the RoPE sperm matmuls (DMA swap), trims row-sum matmuls to
one per (head, i-block), warms the PE clock-gate with dummy matmuls
during the initial DMA wait, and orders/splits the startup DMAs so the
first projection starts ~4us in instead of ~21us.
"""

import numpy as np
import ml_dtypes

import concourse.bass as bass
import concourse.tile as tile
import concourse.mybir as mybir
from concourse import bacc, bass_utils

F32 = mybir.dt.float32
BF16 = mybir.dt.bfloat16

B = 4
S = 2048
D = 2048
NH = 16
DK = 128
NCORES = 8
HPC = 8            # heads per core
DLOC = HPC * DK    # 1024, local model dims per core
ST = S // 128      # 16 sequence 128-tiles
DT = D // 128      # 16 model-dim 128-tiles
NDT = DLOC // 128  # 8 local model-dim 128-tiles
IB = S // 512      # 4 i-blocks of 512
ROPE_THETA = 10000.0
SCALE = float(1.0 / np.sqrt(DK))

_cache = {}


def build_program():
    if "nc" in _cache:
        return _cache["nc"]

    nc = bacc.Bacc("TRN2", target_bir_lowering=False, debug=False,
                   num_devices=NCORES)

    xt = nc.dram_tensor("xt", [4, 128, DT, 512], BF16, kind="ExternalInput").ap()
    wq = nc.dram_tensor("wq", [HPC, 128, DT, DK], BF16, kind="ExternalInput").ap()
    wk = nc.dram_tensor("wk", [HPC, 128, DT, DK], BF16, kind="ExternalInput").ap()
    wv = nc.dram_tensor("wv", [2, 128, DT, 512], BF16, kind="ExternalInput").ap()
    wo = nc.dram_tensor("wo", [4, 128, NDT, 512], BF16, kind="ExternalInput").ap()
    cct = nc.dram_tensor("cct", [128, S], BF16, kind="ExternalInput").ap()
    sst = nc.dram_tensor("sst", [128, S], BF16, kind="ExternalInput").ap()
    tri = nc.dram_tensor("tri", [128, 128], BF16, kind="ExternalInput").ap()
    out = nc.dram_tensor("out", [DT, IB, 128, 512], BF16,
                         kind="ExternalOutput").ap()

    with tile.TileContext(nc) as tc:
        with (
            tc.tile_pool(name="dram", bufs=1, space="DRAM") as dram_pool,
            tc.tile_pool(name="ctx7", bufs=4) as ctx7_pool,
            tc.tile_pool(name="wo0", bufs=1) as wo0_pool,
        ):
            ctx_dram = dram_pool.tile([IB, 128, HPC - 1, 512], BF16)
            wo0_sb = wo0_pool.tile([128, NDT, 512], BF16)
            ctx7 = _attention_phase(nc, tc, xt, wq, wk, wv, cct, sst,
                                    tri, ctx_dram, ctx7_pool, wo, wo0_sb)
            _output_phase(nc, tc, wo, ctx_dram, out, ctx7, wo0_sb)

    nc.compile()
    _cache["nc"] = nc
    return nc


def _attention_phase(nc, tc, xt, wq, wk, wv, cct, sst, tri, ctx_dram,
                     ctx7_pool, wo, wo0_sb):
    with (
        tc.tile_pool(name="xt", bufs=1) as xt_pool,
        tc.tile_pool(name="vsb", bufs=1) as v_pool,
        tc.tile_pool(name="tabs", bufs=1) as tab_pool,
        tc.tile_pool(name="wqk", bufs=3) as wqk_pool,
        tc.tile_pool(name="qkraw", bufs=2) as raw_pool,
        tc.tile_pool(name="rqk", bufs=2) as rqk_pool,
        tc.tile_pool(name="qk_ps", bufs=2, space="PSUM") as qk_ps_pool,
        tc.tile_pool(name="s_ps", bufs=2, space="PSUM") as s_ps_pool,
    ):
        # ---- PE warm-up: the HAM clock gate needs ~3.4us of activity to
        # lift the PE from 1.2 to 2.4 GHz; burn that window with dummy
        # matmuls on a memset tile while the input DMAs run ----
        ones_sb = tab_pool.tile([128, 128], BF16, tag="ones")
        nc.vector.memset(ones_sb[:], 1.0)
        for _ in range(52):
            warm_ps = s_ps_pool.tile([128, 512], F32, tag="s_ps")
            nc.tensor.matmul(warm_ps[:, 0:128], ones_sb[:], ones_sb[:],
                             start=True, stop=True)

        # ---- resident loads (dense contiguous DMAs, ordered so the first
        # projection's dependencies land first) ----
        def load_wqk(h):
            wq_sb = wqk_pool.tile([128, DT, DK], BF16, tag="wq")
            wk_sb = wqk_pool.tile([128, DT, DK], BF16, tag="wk")
            nc.sync.dma_start(wk_sb[:], wk[h])
            nc.sync.dma_start(wq_sb[:], wq[h])
            return wq_sb, wk_sb

        xt_sb = xt_pool.tile([128, 4, DT, 512], BF16)
        wv_sb = tab_pool.tile([128, 2, DT, 512], BF16, tag="wv")
        cc_sb = tab_pool.tile([128, S], BF16, tag="cct")
        ss_sb = tab_pool.tile([128, S], BF16, tag="sst")
        tri_sb = tab_pool.tile([128, 128], BF16, tag="tri")

        # startup DMAs in exact consumption order (single FIFO queue at
        # ~325GB/s; position in the queue IS the arrival time)
        wq_sb0 = wqk_pool.tile([128, DT, DK], BF16, tag="wq")
        wk_sb0 = wqk_pool.tile([128, DT, DK], BF16, tag="wk")
        half = DT // 2
        nc.sync.dma_start(wk_sb0[:, 0:half], wk[0, :, 0:half])
        nc.sync.dma_start(xt_sb[:, 0, 0:4, :], xt[0, :, 0:4, :])
        nc.sync.dma_start(wq_sb0[:, 0:half], wq[0, :, 0:half])
        nc.sync.dma_start(xt_sb[:, 0, 4:8, :], xt[0, :, 4:8, :])
        nc.sync.dma_start(wk_sb0[:, half:DT], wk[0, :, half:DT])
        nc.sync.dma_start(xt_sb[:, 0, 8:12, :], xt[0, :, 8:12, :])
        nc.sync.dma_start(wq_sb0[:, half:DT], wq[0, :, half:DT])
        nc.sync.dma_start(xt_sb[:, 0, 12:16, :], xt[0, :, 12:16, :])
        wqk0 = (wq_sb0, wk_sb0)
        nc.sync.dma_start(tri_sb[:], tri)
        nc.sync.dma_start(cc_sb[:, 0:512], cct[:, 0:512])
        nc.sync.dma_start(ss_sb[:, 0:512], sst[:, 0:512])
        nc.sync.dma_start(wv_sb[:, 0], wv[0])
        nc.sync.dma_start(xt_sb[:, 1], xt[1])
        for ch in range(1, 4):
            o = ch * 512
            nc.sync.dma_start(cc_sb[:, o:o + 512], cct[:, o:o + 512])
            nc.sync.dma_start(ss_sb[:, o:o + 512], sst[:, o:o + 512])
        nc.sync.dma_start(xt_sb[:, 2], xt[2])
        nc.sync.dma_start(wv_sb[:, 1], wv[1])
        nc.sync.dma_start(xt_sb[:, 3], xt[3])
        # first wo ft-group: outer-scope SBUF (no attention-pool aliasing,
        # so no WAR wait pinning it to the end of attention), loaded here
        # right behind the resident inputs on the sync ring
        nc.sync.dma_start(wo0_sb[:], wo[0])

        def proj_chunk(w_sb, r_t, ch):
            o = ch * 512
            ps = qk_ps_pool.tile([128, 512], F32, tag="qk_ps")
            for dt in range(DT):
                nc.tensor.matmul(
                    ps[:],
                    w_sb[:, dt, :],
                    xt_sb[:, ch, dt, :],
                    start=(dt == 0), stop=(dt == DT - 1),
                )
            raw = raw_pool.tile([128, 512], BF16, tag="qkraw")
            nc.scalar.copy(raw[:], ps[:])
            # partition-half rotation via SBUF->SBUF DMA (sign folded
            # into the host-built ss table)
            swp = raw_pool.tile([128, 512], BF16, tag="qkswp")
            nc.scalar.dma_start(swp[0:64, :], raw[64:128, :])
            nc.scalar.dma_start(swp[64:128, :], raw[0:64, :])
            t2 = raw_pool.tile([128, 512], F32, tag="t2")
            nc.vector.tensor_mul(t2[:], swp[:], ss_sb[:, o:o + 512])
            t3 = raw_pool.tile([128, 512], F32, tag="t2")
            nc.vector.tensor_mul(t3[:], raw[:], cc_sb[:, o:o + 512])
            nc.vector.tensor_add(r_t[:, o:o + 512], t2[:], t3[:])

        # head 0's projection is emitted per-chunk, interleaved with its
        # attention i-blocks (chunk ib is exactly what i-block ib consumes),
        # so DMA-paced chunks don't head-of-line-block ready attention work
        rq0 = rqk_pool.tile([128, S], BF16, tag="rq")
        rk0 = rqk_pool.tile([128, S], BF16, tag="rk")
        proj_chunk(wqk0[1], rk0, 0)
        proj_chunk(wqk0[0], rq0, 0)
        rqk0 = (rq0, rk0)

        # ---- V = x @ wv.T (emitted interleaved with head-0 attention) ----
        v_sb = v_pool.tile([128, ST, DLOC], BF16)

        def emit_v(st, g):
            v_ps = qk_ps_pool.tile([128, 512], F32, tag="qk_ps")
            for dt in range(DT):
                nc.tensor.matmul(
                    v_ps[:],
                    xt_sb[:, st // 4, dt, (st % 4) * 128:(st % 4 + 1) * 128],
                    wv_sb[:, g, dt, :],
                    start=(dt == 0), stop=(dt == DT - 1),
                )
            nc.scalar.copy(v_sb[:, st, g * 512:(g + 1) * 512], v_ps[:])

        # ---- per-head attention (+ next head's projection interleaved) ----
        with (
            tc.tile_pool(name="exps", bufs=6) as exp_pool,
            tc.tile_pool(name="pair", bufs=1) as pair_pool,
            tc.tile_pool(name="rsacc", bufs=1) as acc_pool,
            tc.tile_pool(name="small", bufs=1) as small_pool,
            tc.tile_pool(name="ctxsb", bufs=1) as ctx_sb_pool,
            tc.tile_pool(name="ctx_ps", bufs=2, space="PSUM") as ctx_ps_pool,
            tc.tile_pool(name="rs_ps", bufs=2, space="PSUM") as rs_ps_pool,
        ):
            ctx7 = []
            # wqk prefetched two heads ahead (bufs=3) so the loads' WAR
            # waits clear long before the transfers are needed
            wqk_pref = {1: load_wqk(1), 2: load_wqk(2)}
            for h in range(HPC):
                if h == 0:
                    rq, rk = rqk0
                else:
                    if h + 2 < HPC:
                        wqk_pref[h + 2] = load_wqk(h + 2)
                    wqn, wkn = wqk_pref.pop(h)
                    rq = rqk_pool.tile([128, S], BF16, tag="rq")
                    rk = rqk_pool.tile([128, S], BF16, tag="rk")
                    for ch in range(4):
                        proj_chunk(wkn, rk, ch)
                        proj_chunk(wqn, rq, ch)

                for ib in range(IB):
                    if h == 0:
                        if ib > 0:
                            proj_chunk(wqk0[1], rk, ib)
                            proj_chunk(wqk0[0], rq, ib)
                        # V tiles this i-block needs (g=0), just in time
                        for st in range(4 * ib, 4 * ib + 4):
                            emit_v(st, 0)
                    elif h == 1 and ib == 0:
                        for st in range(ST):
                            emit_v(st, 1)
                    i0 = ib * 512
                    ctx_ps = ctx_ps_pool.tile([128, 512], F32, tag="ctx_ps")
                    rs_ps = rs_ps_pool.tile([128, 512], F32, tag="rs_ps")
                    njt = 4 * ib + 4
                    es_prev = None
                    acc = None
                    for jt in range(njt):
                        r = jt - 4 * ib  # >=0 on diagonal blocks
                        lo = 128 * r if r >= 0 else 0
                        s_ps = s_ps_pool.tile([128, 512], F32, tag="s_ps")
                        nc.tensor.matmul(
                            s_ps[:, lo:512],
                            rk[:, jt * 128:(jt + 1) * 128],
                            rq[:, i0 + lo:i0 + 512],
                            start=True, stop=True,
                        )
                        es = exp_pool.tile([128, 512], BF16, tag="exps")
                        nc.scalar.activation(es[:, lo:512], s_ps[:, lo:512],
                                             mybir.ActivationFunctionType.Exp,
                                             scale=SCALE)
                        if r >= 0:
                            nc.vector.tensor_mul(es[:, lo:lo + 128],
                                                 es[:, lo:lo + 128], tri_sb[:])
                            if r >= 1:
                                # zero the never-written cols so diagonal
                                # tiles join the full-width row-sum chain
                                nc.vector.memset(es[:, 0:lo], 0.0)
                        first = (jt == 0)
                        last = (jt == njt - 1)
                        # row sums: all tiles pair-sum on DVE into one
                        # accumulator; a single ones-matmul per i-block
                        # does the partition-axis reduction (output rows
                        # all equal the row sum - free broadcast)
                        if jt % 2 == 0:
                            es_prev = es
                        elif jt == 1:
                            acc = acc_pool.tile([128, 512], BF16, tag="acc")
                            nc.vector.tensor_add(acc[:], es_prev[:], es[:])
                        else:
                            pair = pair_pool.tile([128, 512], BF16,
                                                  tag="pair")
                            nc.vector.tensor_add(pair[:], es_prev[:], es[:])
                            nc.vector.tensor_add(acc[:], acc[:], pair[:])
                        nc.tensor.matmul(
                            ctx_ps[:, lo:512],
                            v_sb[:, jt, h * DK:(h + 1) * DK],
                            es[:, lo:512],
                            start=first, stop=last, skip_group_check=True,
                        )
                    nc.tensor.matmul(
                        rs_ps[:],
                        ones_sb[:],
                        acc[:],
                        start=True, stop=True, skip_group_check=True,
                    )
                    recip = small_pool.tile([128, 512], F32, tag="recip")
                    nc.vector.reciprocal_approx_fast(recip[:], rs_ps[:])
                    if h == HPC - 1:
                        ctx_sb = ctx7_pool.tile([128, 512], BF16, tag="c7")
                        ctx7.append(ctx_sb)
                    else:
                        ctx_sb = ctx_sb_pool.tile([128, 512], BF16,
                                                  tag="ctx_sb")
                    nc.vector.tensor_mul(ctx_sb[:], ctx_ps[:], recip[:])
                    if h != HPC - 1:
                        nc.gpsimd.dma_start(ctx_dram[ib, :, h, :], ctx_sb[:])
            return ctx7


def _output_phase(nc, tc, wo, ctx_dram, out, ctx7, wo0_sb):
    with (
        tc.tile_pool(name="wos", bufs=1) as wo_pool,
        tc.tile_pool(name="ctxin", bufs=4) as cin_pool,
        tc.tile_pool(name="outsb", bufs=3) as out_pool,
        tc.tile_pool(name="wo_ps", bufs=4, space="PSUM") as wo_ps_pool,
    ):
        wo_sb = wo_pool.tile([128, 3, NDT, 512], BF16)
        cins = []
        for sb4 in range(IB):
            cin = cin_pool.tile([128, NDT - 1, 512], BF16, tag="cin")
            if sb4 == 0:
                nc.sync.dma_start(cin[:, 0:4], ctx_dram[0, :, 0:4])
                nc.sync.dma_start(cin[:, 4:7], ctx_dram[0, :, 4:7])
            else:
                nc.sync.dma_start(wo_sb[:, sb4 - 1], wo[sb4])
                nc.sync.dma_start(cin[:], ctx_dram[sb4])
            cins.append(cin)
        for sb4 in range(IB):
            cin = cins[sb4]
            # head 7's contribution comes from SBUF-resident ctx (no DRAM
            # round-trip on the tail); it is the last accumulation per group
            for ft in range(DT):
                ps = wo_ps_pool.tile([128, 512], F32, tag="wo_ps")
                for dt in range(NDT - 1):
                    nc.tensor.matmul(
                        ps[:],
                        (wo0_sb[:, dt] if ft < 4 else
                         wo_sb[:, ft // 4 - 1, dt])[
                            :, (ft % 4) * 128:(ft % 4 + 1) * 128],
                        cin[:, dt, :],
                        start=(dt == 0), stop=False,
                    )
                nc.tensor.matmul(
                    ps[:],
                    (wo0_sb[:, NDT - 1] if ft < 4 else
                     wo_sb[:, ft // 4 - 1, NDT - 1])[
                        :, (ft % 4) * 128:(ft % 4 + 1) * 128],
                    ctx7[sb4][:],
                    start=False, stop=True,
                )
                if sb4 == IB - 1 and ft == DT - 1:
                    # split the final store so the last DMA is small
                    # (shorter post-matmul drain tail)
                    for hh in range(2):
                        osb = out_pool.tile([128, 256], BF16, tag="osbh")
                        nc.scalar.copy(osb[:], ps[:, hh * 256:(hh + 1) * 256])
                        nc.sync.dma_start(
                            out[ft, sb4, :, hh * 256:(hh + 1) * 256], osb[:])
                else:
                    osb = out_pool.tile([128, 512], BF16, tag="osb")
                    nc.scalar.copy(osb[:], ps[:])
                    nc.sync.dma_start(out[ft, sb4], osb[:])


def _tile2(a, p, q):
    """[R, C] -> [R//p, C//q, p, q] contiguous blocks."""
    R, C = a.shape
    return np.ascontiguousarray(
        a.reshape(R // p, p, C // q, q).transpose(0, 2, 1, 3))


def prepare_in_maps(x, wq, wk, wv, wo):
    """Build the 8 per-core input maps (host-side sharding + tables)."""
    x = np.asarray(x, dtype=np.float32)
    wq = np.asarray(wq, dtype=np.float32)
    wk = np.asarray(wk, dtype=np.float32)
    wv = np.asarray(wv, dtype=np.float32)
    wo = np.asarray(wo, dtype=np.float32)
    bf16 = ml_dtypes.bfloat16

    # RoPE tables (fp32, matching the reference's fp32 cos/sin); the
    # pair-rotation sign is folded into ss (swap is an unsigned
    # partition-half rotation on device)
    f = np.arange(0, DK, 2, dtype=np.float32) / DK          # 2f/d
    inv_freq = (ROPE_THETA ** (-f)).astype(np.float32)      # [64]
    ang = np.arange(S, dtype=np.float32)[:, None] * inv_freq[None, :]
    cos_t = np.cos(ang).T.astype(np.float32)                # [64, S]
    sin_t = np.sin(ang).T.astype(np.float32)
    cc = np.ascontiguousarray(np.vstack([cos_t, cos_t])).astype(bf16)
    ss = np.ascontiguousarray(np.vstack([-sin_t, sin_t])).astype(bf16)

    tri = np.tril(np.ones((128, 128), dtype=np.float32)).T  # tri[j,i]=1 if j<=i
    tri = np.ascontiguousarray(tri).astype(bf16)

    deint = np.concatenate([np.arange(0, DK, 2), np.arange(1, DK, 2)])
    in_maps = []
    for c in range(NCORES):
        bi, g = divmod(c, 2)
        heads = [g * HPC + h for h in range(HPC)]
        qk_rows = np.concatenate([hg * DK + deint for hg in heads])
        v_rows = np.arange(g * DLOC, (g + 1) * DLOC)

        # xt [4ch, 128p, DT, 512c]: value = x[bi].T[dt*128+p, ch*512+c]
        xtT = x[bi].T.astype(bf16)                              # [d, s]
        xt_t = xtT.reshape(DT, 128, 4, 512).transpose(2, 1, 0, 3)
        # wq/wk [HPC, 128p, DT, DK]: value = w.T[dt*128+p, h*DK+k]
        wq_t = _tile2(wq[qk_rows, :].T.astype(bf16), 128, DK) \
            .transpose(1, 2, 0, 3)
        wk_t = _tile2(wk[qk_rows, :].T.astype(bf16), 128, DK) \
            .transpose(1, 2, 0, 3)
        # wv [2g, 128p, DT, 512]: value = wv.T[dt*128+p, g*512+c]
        wv_t = _tile2(wv[v_rows, :].T.astype(bf16), 128, 512) \
            .transpose(1, 2, 0, 3)
        # wo [4ftg, 128p, NDT, 512]: value = wo.T[v_rows][dt*128+p, f]
        wo_t = wo.T[v_rows, :].astype(bf16).reshape(NDT, 128, 4, 512) \
            .transpose(2, 1, 0, 3)
        in_maps.append({
            "xt": np.ascontiguousarray(xt_t),
            "wq": np.ascontiguousarray(wq_t),
            "wk": np.ascontiguousarray(wk_t),
            "wv": np.ascontiguousarray(wv_t),
            "wo": np.ascontiguousarray(wo_t),
            "cct": cc, "sst": ss,
            "tri": tri,
        })
    return in_maps


def assemble(results):
    out = np.empty((B, S, D), dtype=np.float32)
    for bi in range(B):
        oT = (results[2 * bi]["out"].astype(np.float32)
              + results[2 * bi + 1]["out"].astype(np.float32))
        # oT: [DT, IB, 128, 512] -> out^T [f, s]; out[b] = out^T.T
        oT = oT.transpose(0, 2, 1, 3).reshape(D, S)
        out[bi] = oT.T
    return out


def kernel(**inputs):
    nc = build_program()
    in_maps = prepare_in_maps(inputs["x"], inputs["wq"], inputs["wk"],
                              inputs["wv"], inputs["wo"])
    res = bass_utils.run_bass_kernel_spmd(nc, in_maps,
                                          core_ids=list(range(NCORES)))
    return assemble(res.results)


# revision 25
# speedup vs baseline: 1.1869x; 1.1869x over previous
"""Multi-head self-attention (RoPE, causal) on 8 Trainium2 NeuronCores.

Sharding: core c -> (batch = c//2, head-group = c%2 of 8 heads).
Column-parallel wq/wk/wv, row-parallel wo. Each core emits a partial
out^T [f, s]; the host sums the two partials per batch and transposes.

Layouts (all chosen so no on-device transposes are needed):
  XT  [d, s]   (x transposed on host, bf16)
  Q^T/K^T [e, s] per head from matmul(lhsT=wT[d,e], rhs=XT[d,s])
  V   [s, e]   from matmul(lhsT=XT[d,s], rhs=wvT[d,e])
  S^T [j, i] = matmul(lhsT=K^T[e,j], rhs=Q^T[e,i])
  ctx^T [e, i] = matmul(lhsT=V[j,e], rhs=expS^T[j,i])
  out^T [f, s] = matmul(lhsT=woT[d,f], rhs=ctx^T[d,s])

All DRAM inputs are pre-tiled on the host so every DMA moves dense,
multi-KB contiguous per-partition lines (<=2KB lines run at ~1/2 DMA
rate). All matmul operands are bf16 (PSUM accumulation stays fp32);
softmax statistics and RoPE arithmetic stay fp32.

RoPE: head dims de-interleaved on host (even dims -> partitions 0..63,
odd -> 64..127 of each head's Q^T/K^T) by permuting wq/wk rows. Then
rot(x) = x*cc + swap(x)*ss_signed where swap is a partition-half
rotation done by two SBUF->SBUF DMAs (free on the tensor engine) and
the pair-rotation sign lives in the host-built ss table. The
1/sqrt(dk) scale is applied via the Exp activation's scale field.

Softmax: no max-subtraction (scores are O(1)-scaled; fp32 exp is safe).
Causal masking by block-skipping + one 128x128 triangular mask on
diagonal blocks. Row sums via an all-ones [128,128] matmul on a
DVE-accumulated sum of the i-block's exp tiles (output rows all equal
the row sum, giving the partition broadcast for free); normalization
multiplies ctx^T by a fast DVE reciprocal of that tile.

The tensor engine is the bottleneck (~94% busy): ~2700 matmul
instructions at the 512-column bf16 streaming rate. This version
removes the RoPE sperm matmuls (DMA swap), trims row-sum matmuls to
one per (head, i-block), warms the PE clock-gate with dummy matmuls
during the initial DMA wait, and orders/splits the startup DMAs so the
first projection starts ~4us in instead of ~21us.
"""

import numpy as np
import ml_dtypes

import concourse.bass as bass
import concourse.tile as tile
import concourse.mybir as mybir
from concourse import bacc, bass_utils

F32 = mybir.dt.float32
BF16 = mybir.dt.bfloat16

B = 4
S = 2048
D = 2048
NH = 16
DK = 128
NCORES = 8
HPC = 8            # heads per core
DLOC = HPC * DK    # 1024, local model dims per core
ST = S // 128      # 16 sequence 128-tiles
DT = D // 128      # 16 model-dim 128-tiles
NDT = DLOC // 128  # 8 local model-dim 128-tiles
IB = S // 512      # 4 i-blocks of 512
ROPE_THETA = 10000.0
SCALE = float(1.0 / np.sqrt(DK))

_cache = {}


def build_program():
    if "nc" in _cache:
        return _cache["nc"]

    nc = bacc.Bacc("TRN2", target_bir_lowering=False, debug=False,
                   num_devices=NCORES)

    xt = nc.dram_tensor("xt", [4, 128, DT, 512], BF16, kind="ExternalInput").ap()
    wq = nc.dram_tensor("wq", [HPC, 128, DT, DK], BF16, kind="ExternalInput").ap()
    wk = nc.dram_tensor("wk", [HPC, 128, DT, DK], BF16, kind="ExternalInput").ap()
    wv = nc.dram_tensor("wv", [2, 128, DT, 512], BF16, kind="ExternalInput").ap()
    wo = nc.dram_tensor("wo", [4, 128, NDT, 512], BF16, kind="ExternalInput").ap()
    cct = nc.dram_tensor("cct", [128, S], BF16, kind="ExternalInput").ap()
    sst = nc.dram_tensor("sst", [128, S], BF16, kind="ExternalInput").ap()
    tri = nc.dram_tensor("tri", [128, 512], BF16, kind="ExternalInput").ap()
    out = nc.dram_tensor("out", [DT, IB, 128, 512], BF16,
                         kind="ExternalOutput").ap()

    with tile.TileContext(nc) as tc:
        with (
            tc.tile_pool(name="dram", bufs=1, space="DRAM") as dram_pool,
            tc.tile_pool(name="ctx7", bufs=4) as ctx7_pool,
            tc.tile_pool(name="wo0", bufs=1) as wo0_pool,
        ):
            ctx_dram = dram_pool.tile([IB, 128, HPC - 1, 512], BF16)
            wo0_sb = wo0_pool.tile([128, NDT, 512], BF16)
            ctx7 = _attention_phase(nc, tc, xt, wq, wk, wv, cct, sst,
                                    tri, ctx_dram, ctx7_pool, wo, wo0_sb)
            _output_phase(nc, tc, wo, ctx_dram, out, ctx7, wo0_sb)

    nc.compile()
    _cache["nc"] = nc
    return nc


def _attention_phase(nc, tc, xt, wq, wk, wv, cct, sst, tri, ctx_dram,
                     ctx7_pool, wo, wo0_sb):
    with (
        tc.tile_pool(name="xt", bufs=1) as xt_pool,
        tc.tile_pool(name="vsb", bufs=1) as v_pool,
        tc.tile_pool(name="tabs", bufs=1) as tab_pool,
        tc.tile_pool(name="wqk", bufs=3) as wqk_pool,
        tc.tile_pool(name="qkraw", bufs=2) as raw_pool,
        tc.tile_pool(name="rqk", bufs=2) as rqk_pool,
        tc.tile_pool(name="qk_ps", bufs=2, space="PSUM") as qk_ps_pool,
        tc.tile_pool(name="s_ps", bufs=2, space="PSUM") as s_ps_pool,
    ):
        # ---- PE warm-up: the HAM clock gate needs ~3.4us of activity to
        # lift the PE from 1.2 to 2.4 GHz; burn that window with dummy
        # matmuls on a memset tile while the input DMAs run ----
        ones_sb = tab_pool.tile([128, 128], BF16, tag="ones")
        nc.vector.memset(ones_sb[:], 1.0)
        for _ in range(52):
            warm_ps = s_ps_pool.tile([128, 512], F32, tag="s_ps")
            nc.tensor.matmul(warm_ps[:, 0:128], ones_sb[:], ones_sb[:],
                             start=True, stop=True)

        # ---- resident loads (dense contiguous DMAs, ordered so the first
        # projection's dependencies land first) ----
        def load_wqk(h):
            wq_sb = wqk_pool.tile([128, DT, DK], BF16, tag="wq")
            wk_sb = wqk_pool.tile([128, DT, DK], BF16, tag="wk")
            nc.sync.dma_start(wk_sb[:], wk[h])
            nc.sync.dma_start(wq_sb[:], wq[h])
            return wq_sb, wk_sb

        xt_sb = xt_pool.tile([128, 4, DT, 512], BF16)
        wv_sb = tab_pool.tile([128, 2, DT, 512], BF16, tag="wv")
        cc_sb = tab_pool.tile([128, S], BF16, tag="cct")
        ss_sb = tab_pool.tile([128, S], BF16, tag="sst")
        tri_sb = tab_pool.tile([128, 512], BF16, tag="tri")

        # startup DMAs in exact consumption order (single FIFO queue at
        # ~325GB/s; position in the queue IS the arrival time)
        wq_sb0 = wqk_pool.tile([128, DT, DK], BF16, tag="wq")
        wk_sb0 = wqk_pool.tile([128, DT, DK], BF16, tag="wk")
        half = DT // 2
        nc.sync.dma_start(wk_sb0[:, 0:half], wk[0, :, 0:half])
        nc.sync.dma_start(xt_sb[:, 0, 0:4, :], xt[0, :, 0:4, :])
        nc.sync.dma_start(wq_sb0[:, 0:half], wq[0, :, 0:half])
        nc.sync.dma_start(xt_sb[:, 0, 4:8, :], xt[0, :, 4:8, :])
        nc.sync.dma_start(wk_sb0[:, half:DT], wk[0, :, half:DT])
        nc.sync.dma_start(xt_sb[:, 0, 8:12, :], xt[0, :, 8:12, :])
        nc.sync.dma_start(wq_sb0[:, half:DT], wq[0, :, half:DT])
        nc.sync.dma_start(xt_sb[:, 0, 12:16, :], xt[0, :, 12:16, :])
        wqk0 = (wq_sb0, wk_sb0)
        nc.sync.dma_start(tri_sb[:], tri)
        nc.sync.dma_start(cc_sb[:, 0:512], cct[:, 0:512])
        nc.sync.dma_start(ss_sb[:, 0:512], sst[:, 0:512])
        nc.sync.dma_start(wv_sb[:, 0], wv[0])
        nc.sync.dma_start(xt_sb[:, 1], xt[1])
        for ch in range(1, 4):
            o = ch * 512
            nc.sync.dma_start(cc_sb[:, o:o + 512], cct[:, o:o + 512])
            nc.sync.dma_start(ss_sb[:, o:o + 512], sst[:, o:o + 512])
        nc.sync.dma_start(xt_sb[:, 2], xt[2])
        nc.sync.dma_start(wv_sb[:, 1], wv[1])
        nc.sync.dma_start(xt_sb[:, 3], xt[3])
        # first wo ft-group: outer-scope SBUF (no attention-pool aliasing,
        # so no WAR wait pinning it to the end of attention), loaded here
        # right behind the resident inputs on the sync ring
        nc.sync.dma_start(wo0_sb[:], wo[0])

        def proj_chunk(w_sb, r_t, ch):
            o = ch * 512
            ps = qk_ps_pool.tile([128, 512], F32, tag="qk_ps")
            for dt in range(DT):
                nc.tensor.matmul(
                    ps[:],
                    w_sb[:, dt, :],
                    xt_sb[:, ch, dt, :],
                    start=(dt == 0), stop=(dt == DT - 1),
                )
            raw = raw_pool.tile([128, 512], BF16, tag="qkraw")
            nc.scalar.copy(raw[:], ps[:])
            # partition-half rotation via SBUF->SBUF DMA (sign folded
            # into the host-built ss table)
            swp = raw_pool.tile([128, 512], BF16, tag="qkswp")
            nc.scalar.dma_start(swp[0:64, :], raw[64:128, :])
            nc.scalar.dma_start(swp[64:128, :], raw[0:64, :])
            t2 = raw_pool.tile([128, 512], F32, tag="t2")
            nc.vector.tensor_mul(t2[:], swp[:], ss_sb[:, o:o + 512])
            t3 = raw_pool.tile([128, 512], F32, tag="t2")
            nc.vector.tensor_mul(t3[:], raw[:], cc_sb[:, o:o + 512])
            nc.vector.tensor_add(r_t[:, o:o + 512], t2[:], t3[:])

        # head 0's projection is emitted per-chunk, interleaved with its
        # attention i-blocks (chunk ib is exactly what i-block ib consumes),
        # so DMA-paced chunks don't head-of-line-block ready attention work
        rq0 = rqk_pool.tile([128, S], BF16, tag="rq")
        rk0 = rqk_pool.tile([128, S], BF16, tag="rk")
        proj_chunk(wqk0[1], rk0, 0)
        proj_chunk(wqk0[0], rq0, 0)
        rqk0 = (rq0, rk0)

        # ---- V = x @ wv.T (emitted interleaved with head-0 attention) ----
        v_sb = v_pool.tile([128, ST, DLOC], BF16)

        def emit_v(st, g):
            v_ps = qk_ps_pool.tile([128, 512], F32, tag="qk_ps")
            for dt in range(DT):
                nc.tensor.matmul(
                    v_ps[:],
                    xt_sb[:, st // 4, dt, (st % 4) * 128:(st % 4 + 1) * 128],
                    wv_sb[:, g, dt, :],
                    start=(dt == 0), stop=(dt == DT - 1),
                )
            nc.scalar.copy(v_sb[:, st, g * 512:(g + 1) * 512], v_ps[:])

        # ---- per-head attention (+ next head's projection interleaved) ----
        with (
            tc.tile_pool(name="exps", bufs=5) as exp_pool,
            tc.tile_pool(name="pair", bufs=1) as pair_pool,
            tc.tile_pool(name="rsacc", bufs=1) as acc_pool,
            tc.tile_pool(name="small", bufs=1) as small_pool,
            tc.tile_pool(name="ctxsb", bufs=1) as ctx_sb_pool,
            tc.tile_pool(name="ctx_ps", bufs=2, space="PSUM") as ctx_ps_pool,
            tc.tile_pool(name="rs_ps", bufs=2, space="PSUM") as rs_ps_pool,
        ):
            for _ in range(5):
                es0 = exp_pool.tile([128, 512], BF16, tag="exps")
                nc.gpsimd.memset(es0[:], 0.0)
            ctx7 = []
            # wqk prefetched two heads ahead (bufs=3) so the loads' WAR
            # waits clear long before the transfers are needed
            wqk_pref = {1: load_wqk(1), 2: load_wqk(2)}
            for h in range(HPC):
                if h == 0:
                    rq, rk = rqk0
                else:
                    if h + 2 < HPC:
                        wqk_pref[h + 2] = load_wqk(h + 2)
                    wqn, wkn = wqk_pref.pop(h)
                    rq = rqk_pool.tile([128, S], BF16, tag="rq")
                    rk = rqk_pool.tile([128, S], BF16, tag="rk")
                    for ch in range(4):
                        proj_chunk(wkn, rk, ch)
                        proj_chunk(wqn, rq, ch)

                for ib in range(IB):
                    if h == 0:
                        if ib > 0:
                            proj_chunk(wqk0[1], rk, ib)
                            proj_chunk(wqk0[0], rq, ib)
                        # V tiles this i-block needs (g=0), just in time
                        for st in range(4 * ib, 4 * ib + 4):
                            emit_v(st, 0)
                    elif h == 1 and ib == 0:
                        for st in range(ST):
                            emit_v(st, 1)
                    i0 = ib * 512
                    ctx_ps = ctx_ps_pool.tile([128, 512], F32, tag="ctx_ps")
                    rs_ps = rs_ps_pool.tile([128, 512], F32, tag="rs_ps")
                    njt = 4 * ib + 4
                    es_prev = None
                    acc = None
                    for jt in range(njt):
                        r = jt - 4 * ib  # >=0 on diagonal blocks
                        lo = 128 * r if r >= 0 else 0
                        s_ps = s_ps_pool.tile([128, 512], F32, tag="s_ps")
                        nc.tensor.matmul(
                            s_ps[:, lo:512],
                            rk[:, jt * 128:(jt + 1) * 128],
                            rq[:, i0 + lo:i0 + 512],
                            start=True, stop=True,
                        )
                        es = exp_pool.tile([128, 512], BF16, tag="exps")
                        nc.scalar.activation(es[:, lo:512], s_ps[:, lo:512],
                                             mybir.ActivationFunctionType.Exp,
                                             scale=SCALE)
                        if r >= 0:
                            # one widened multiply masks the boundary block
                            # AND zeroes the never-written cols, so diagonal
                            # tiles join the full-width row-sum chain
                            # (ztri = [zeros(384) | tril^T])
                            nc.vector.tensor_mul(
                                es[:, 0:lo + 128], es[:, 0:lo + 128],
                                tri_sb[:, 384 - lo:512])
                        first = (jt == 0)
                        last = (jt == njt - 1)
                        # row sums: all tiles pair-sum on DVE into one
                        # accumulator; a single ones-matmul per i-block
                        # does the partition-axis reduction (output rows
                        # all equal the row sum - free broadcast)
                        if jt % 2 == 0:
                            es_prev = es
                        elif jt == 1:
                            acc = acc_pool.tile([128, 512], BF16, tag="acc")
                            nc.vector.tensor_add(acc[:], es_prev[:], es[:])
                        else:
                            pair = pair_pool.tile([128, 512], BF16,
                                                  tag="pair")
                            nc.vector.tensor_add(pair[:], es_prev[:], es[:])
                            nc.vector.tensor_add(acc[:], acc[:], pair[:])
                        nc.tensor.matmul(
                            ctx_ps[:, lo:512],
                            v_sb[:, jt, h * DK:(h + 1) * DK],
                            es[:, lo:512],
                            start=first, stop=last, skip_group_check=True,
                        )
                    nc.tensor.matmul(
                        rs_ps[:],
                        ones_sb[:],
                        acc[:],
                        start=True, stop=True, skip_group_check=True,
                    )
                    recip = small_pool.tile([128, 512], F32, tag="recip")
                    nc.vector.reciprocal_approx_fast(recip[:], rs_ps[:])
                    if h == HPC - 1:
                        ctx_sb = ctx7_pool.tile([128, 512], BF16, tag="c7")
                        ctx7.append(ctx_sb)
                    else:
                        ctx_sb = ctx_sb_pool.tile([128, 512], BF16,
                                                  tag="ctx_sb")
                    nc.vector.tensor_mul(ctx_sb[:], ctx_ps[:], recip[:])
                    if h != HPC - 1:
                        nc.gpsimd.dma_start(ctx_dram[ib, :, h, :], ctx_sb[:])
            return ctx7


def _output_phase(nc, tc, wo, ctx_dram, out, ctx7, wo0_sb):
    with (
        tc.tile_pool(name="wos", bufs=1) as wo_pool,
        tc.tile_pool(name="ctxin", bufs=4) as cin_pool,
        tc.tile_pool(name="outsb", bufs=3) as out_pool,
        tc.tile_pool(name="wo_ps", bufs=4, space="PSUM") as wo_ps_pool,
    ):
        wo_sb = wo_pool.tile([128, 3, NDT, 512], BF16)
        cins = []
        for sb4 in range(IB):
            cin = cin_pool.tile([128, NDT - 1, 512], BF16, tag="cin")
            if sb4 == 0:
                nc.sync.dma_start(cin[:, 0:4], ctx_dram[0, :, 0:4])
                nc.sync.dma_start(cin[:, 4:7], ctx_dram[0, :, 4:7])
            else:
                nc.sync.dma_start(wo_sb[:, sb4 - 1], wo[sb4])
                nc.sync.dma_start(cin[:], ctx_dram[sb4])
            cins.append(cin)
        for sb4 in range(IB):
            cin = cins[sb4]
            # head 7's contribution comes from SBUF-resident ctx (no DRAM
            # round-trip on the tail); it is the last accumulation per group
            for ft in range(DT):
                ps = wo_ps_pool.tile([128, 512], F32, tag="wo_ps")
                for dt in range(NDT - 1):
                    nc.tensor.matmul(
                        ps[:],
                        (wo0_sb[:, dt] if ft < 4 else
                         wo_sb[:, ft // 4 - 1, dt])[
                            :, (ft % 4) * 128:(ft % 4 + 1) * 128],
                        cin[:, dt, :],
                        start=(dt == 0), stop=False,
                    )
                nc.tensor.matmul(
                    ps[:],
                    (wo0_sb[:, NDT - 1] if ft < 4 else
                     wo_sb[:, ft // 4 - 1, NDT - 1])[
                        :, (ft % 4) * 128:(ft % 4 + 1) * 128],
                    ctx7[sb4][:],
                    start=False, stop=True,
                )
                if sb4 == IB - 1 and ft == DT - 1:
                    # split the final store so the last DMA is small
                    # (shorter post-matmul drain tail)
                    for hh in range(2):
                        osb = out_pool.tile([128, 256], BF16, tag="osbh")
                        nc.scalar.copy(osb[:], ps[:, hh * 256:(hh + 1) * 256])
                        nc.sync.dma_start(
                            out[ft, sb4, :, hh * 256:(hh + 1) * 256], osb[:])
                else:
                    osb = out_pool.tile([128, 512], BF16, tag="osb")
                    nc.scalar.copy(osb[:], ps[:])
                    nc.sync.dma_start(out[ft, sb4], osb[:])


def _tile2(a, p, q):
    """[R, C] -> [R//p, C//q, p, q] contiguous blocks."""
    R, C = a.shape
    return np.ascontiguousarray(
        a.reshape(R // p, p, C // q, q).transpose(0, 2, 1, 3))


def prepare_in_maps(x, wq, wk, wv, wo):
    """Build the 8 per-core input maps (host-side sharding + tables)."""
    x = np.asarray(x, dtype=np.float32)
    wq = np.asarray(wq, dtype=np.float32)
    wk = np.asarray(wk, dtype=np.float32)
    wv = np.asarray(wv, dtype=np.float32)
    wo = np.asarray(wo, dtype=np.float32)
    bf16 = ml_dtypes.bfloat16

    # RoPE tables (fp32, matching the reference's fp32 cos/sin); the
    # pair-rotation sign is folded into ss (swap is an unsigned
    # partition-half rotation on device)
    f = np.arange(0, DK, 2, dtype=np.float32) / DK          # 2f/d
    inv_freq = (ROPE_THETA ** (-f)).astype(np.float32)      # [64]
    ang = np.arange(S, dtype=np.float32)[:, None] * inv_freq[None, :]
    cos_t = np.cos(ang).T.astype(np.float32)                # [64, S]
    sin_t = np.sin(ang).T.astype(np.float32)
    cc = np.ascontiguousarray(np.vstack([cos_t, cos_t])).astype(bf16)
    ss = np.ascontiguousarray(np.vstack([-sin_t, sin_t])).astype(bf16)

    # ztri [128, 512]: zeros(384) then tri^T (tri[j,i]=1 if j<=i); slicing
    # ztri[:, 384-lo:512] gives [zeros(lo) | tri] for the diagonal mask
    tri = np.zeros((128, 512), dtype=np.float32)
    tri[:, 384:512] = np.tril(np.ones((128, 128), dtype=np.float32)).T
    tri = np.ascontiguousarray(tri).astype(bf16)

    deint = np.concatenate([np.arange(0, DK, 2), np.arange(1, DK, 2)])
    in_maps = []
    for c in range(NCORES):
        bi, g = divmod(c, 2)
        heads = [g * HPC + h for h in range(HPC)]
        qk_rows = np.concatenate([hg * DK + deint for hg in heads])
        v_rows = np.arange(g * DLOC, (g + 1) * DLOC)

        # xt [4ch, 128p, DT, 512c]: value = x[bi].T[dt*128+p, ch*512+c]
        xtT = x[bi].T.astype(bf16)                              # [d, s]
        xt_t = xtT.reshape(DT, 128, 4, 512).transpose(2, 1, 0, 3)
        # wq/wk [HPC, 128p, DT, DK]: value = w.T[dt*128+p, h*DK+k]
        wq_t = _tile2(wq[qk_rows, :].T.astype(bf16), 128, DK) \
            .transpose(1, 2, 0, 3)
        wk_t = _tile2(wk[qk_rows, :].T.astype(bf16), 128, DK) \
            .transpose(1, 2, 0, 3)
        # wv [2g, 128p, DT, 512]: value = wv.T[dt*128+p, g*512+c]
        wv_t = _tile2(wv[v_rows, :].T.astype(bf16), 128, 512) \
            .transpose(1, 2, 0, 3)
        # wo [4ftg, 128p, NDT, 512]: value = wo.T[v_rows][dt*128+p, f]
        wo_t = wo.T[v_rows, :].astype(bf16).reshape(NDT, 128, 4, 512) \
            .transpose(2, 1, 0, 3)
        in_maps.append({
            "xt": np.ascontiguousarray(xt_t),
            "wq": np.ascontiguousarray(wq_t),
            "wk": np.ascontiguousarray(wk_t),
            "wv": np.ascontiguousarray(wv_t),
            "wo": np.ascontiguousarray(wo_t),
            "cct": cc, "sst": ss,
            "tri": tri,
        })
    return in_maps


def assemble(results):
    out = np.empty((B, S, D), dtype=np.float32)
    for bi in range(B):
        oT = (results[2 * bi]["out"].astype(np.float32)
              + results[2 * bi + 1]["out"].astype(np.float32))
        # oT: [DT, IB, 128, 512] -> out^T [f, s]; out[b] = out^T.T
        oT = oT.transpose(0, 2, 1, 3).reshape(D, S)
        out[bi] = oT.T
    return out


def kernel(**inputs):
    nc = build_program()
    in_maps = prepare_in_maps(inputs["x"], inputs["wq"], inputs["wk"],
                              inputs["wv"], inputs["wo"])
    res = bass_utils.run_bass_kernel_spmd(nc, in_maps,
                                          core_ids=list(range(NCORES)))
    return assemble(res.results)
